# revision 30
# baseline (speedup 1.0000x reference)
"""Causal multi-head attention block (b=8, s=1024, d_model=768, 12 heads x 64)
on 8 TRN2 NeuronCores — batch-parallel: core i computes batch element i.

Self-contained: includes the NTFF-profile-hook shim and the BIR wait-split
workaround for this walrus build (max 1 semaphore wait per instruction).

Per-core plan (bf16 matmuls, fp32 PSUM accumulation):
  A. x arrives bf16 (host-converted); DMA-transpose -> xT tiles [m][128,1024]
  B. QT/KT [hd-blk][128,1024] = W.T @ xT (head-pair packed); V in natural
     [s,hd] layout padded to 65 cols/head with a ones column (rowsum trick)
  C. per q-half(512 = 2 q-chunks) / head: scoresT[k,q] = KT.T @ QT on PE,
     exp on ACT (1/8 scale folded in), causal mask via gpsimd affine_select
     on diagonal k-tiles, PV accumulates [65,256]x2 in one PSUM bank
     (row 64 = softmax denominator); denominators batch through one DVE
     reciprocal per q-half; normalize into the [hd,q] stack; out-proj +
     b_O; DMA out.
"""

import os
import sys
import types

import numpy as np

# ---------------------------------------------------------------------------
# environment shims


def _install_ntff_hook():
    try:
        import antenv
        from trn_agent_boot.trn_boot import _ntff_profile_via_ctypes
    except Exception:
        return
    if "antenv.axon_hooks" in sys.modules:
        return
    hook = _ntff_profile_via_ctypes("/opt/axon/libaxon_pjrt.so")
    m = types.ModuleType("antenv.axon_hooks")
    m.set_axon_ntff_profile_hook = lambda h: None
    m.get_axon_ntff_profile_hook = lambda: hook
    sys.modules["antenv.axon_hooks"] = m
    antenv.axon_hooks = m


def _install_waitsplit(max_waits=1):
    """walrus on this build rejects >1 sem wait per instruction; split extras
    onto preceding NoOps (same engine, program order preserved)."""
    import json

    import concourse.bass as bass

    if getattr(bass.Bass, "_waitsplit_installed", False):
        return
    counter = [0]

    def _split(inst):
        si = inst.get("sync_info")
        if not si:
            return [inst]
        waits = si.get("on_wait") or []
        if len(waits) <= max_waits:
            return [inst]
        out = []
        head, rest = waits[:-max_waits], waits[-max_waits:]
        for i in range(0, len(head), max_waits):
            counter[0] += 1
            out.append(
                {
                    "debug": inst.get("debug", 0),
                    "engine": inst["engine"],
                    "ins": [],
                    "name": f"I-waitsplit-{counter[0]}",
                    "opcode": "NoOp",
                    "outs": [],
                    "text_hint": "waitsplit",
                    "sync_info": {
                        "on_update": [],
                        "on_wait": head[i : i + max_waits],
                    },
                }
            )
        si["on_wait"] = rest
        out.append(inst)
        return out

    orig = bass.Bass.to_json_bytes

    def to_json_bytes(self):
        d = json.loads(orig(self))
        changed = False
        for f in d.get("functions", []):
            for bb in f.get("blocks", []):
                new = []
                for inst in bb.get("instructions", []):
                    parts = _split(inst)
                    changed = changed or len(parts) > 1
                    new.extend(parts)
                bb["instructions"] = new
        return json.dumps(d).encode() if changed else orig(self)

    bass.Bass.to_json_bytes = to_json_bytes
    bass.Bass._waitsplit_installed = True


_install_ntff_hook()
_install_waitsplit()

import ml_dtypes  # noqa: E402
import concourse.bass as bass  # noqa: E402
import concourse.mybir as mybir  # noqa: E402
import concourse.tile as tile  # noqa: E402
from concourse.bass_utils import run_bass_kernel_spmd  # noqa: E402

# ---------------------------------------------------------------------------
# problem constants (hardcoded per harness contract)

B, S, D, H, DH = 8, 1024, 768, 12, 64
P = 128
MT = D // P            # 6 tiles over d_model / hd
QC = 256               # q-chunk width
QH = 512               # q-half (pair of chunks)
NKT = S // P           # 8 k-tiles over seq
SCALE = float(1.0 / np.sqrt(DH))
N_CORES = 8

F32 = mybir.dt.float32
F32R = mybir.dt.float32r
BF16 = mybir.dt.bfloat16
MMDT = BF16


def build_nc() -> bass.Bass:
    nc = bass.Bass()
    xT = nc.declare_dram_parameter("xT", [D, S], MMDT, isOutput=False)
    wq = nc.declare_dram_parameter("wq", [D, D], MMDT, isOutput=False)
    wk = nc.declare_dram_parameter("wk", [D, D], MMDT, isOutput=False)
    wv = nc.declare_dram_parameter("wv", [D, D], MMDT, isOutput=False)
    wo = nc.declare_dram_parameter("wo", [D, D], MMDT, isOutput=False)
    bq = nc.declare_dram_parameter("bq", [D], F32, isOutput=False)
    bk = nc.declare_dram_parameter("bk", [D], F32, isOutput=False)
    bv = nc.declare_dram_parameter("bv", [D], F32, isOutput=False)
    bo = nc.declare_dram_parameter("bo", [D], F32, isOutput=False)
    indsel = nc.declare_dram_parameter("indsel", [H, D], F32, isOutput=False)
    y = nc.declare_dram_parameter("y", [S, D], F32, isOutput=True)

    Exp = mybir.ActivationFunctionType.Exp
    Ident = mybir.ActivationFunctionType.Identity
    mult = mybir.AluOpType.mult
    add = mybir.AluOpType.add
    is_ge = mybir.AluOpType.is_ge

    from contextlib import ExitStack

    with ExitStack() as _ctx:
        tc = _ctx.enter_context(tile.TileContext(nc))
        constp = _ctx.enter_context(tc.tile_pool(name="const", bufs=1))
        xtp = _ctx.enter_context(tc.tile_pool(name="xT", bufs=1))
        qtp = _ctx.enter_context(tc.tile_pool(name="qt", bufs=1))
        ktp = _ctx.enter_context(tc.tile_pool(name="kt", bufs=1))
        vpp = _ctx.enter_context(tc.tile_pool(name="vp", bufs=1))
        wtsp = _ctx.enter_context(tc.tile_pool(name="wts", bufs=24))
        expp = _ctx.enter_context(tc.tile_pool(name="expst", bufs=4))
        wsp = _ctx.enter_context(tc.tile_pool(name="wstack", bufs=12))
        outp = _ctx.enter_context(tc.tile_pool(name="outsb", bufs=2))
        smallp = _ctx.enter_context(tc.tile_pool(name="small", bufs=2))
        psflow = _ctx.enter_context(
            tc.tile_pool(name="ps_flow", bufs=4, space="PSUM")
        )
        psacc = _ctx.enter_context(
            tc.tile_pool(name="ps_acc", bufs=4, space="PSUM")
        )

        # ---- constants -----------------------------------------------------
        bq_t = constp.tile([P, MT], F32, tag="bq")  # col hdb = bias block
        bk_t = constp.tile([P, MT], F32, tag="bk")
        for hdb in range(MT):
            nc.gpsimd.dma_start(
                bq_t[:, hdb : hdb + 1],
                bq[hdb * P : (hdb + 1) * P].rearrange("(p o) -> p o", o=1),
            )
            nc.gpsimd.dma_start(
                bk_t[:, hdb : hdb + 1],
                bk[hdb * P : (hdb + 1) * P].rearrange("(p o) -> p o", o=1),
            )
        ones_stage = constp.tile([1, P], F32, tag="onesstage")
        nc.vector.memset(ones_stage[:], 1.0)
        ones_row = constp.tile([1, P], F32R, tag="onesrow")
        nc.vector.tensor_copy(ones_row[:], ones_stage[:])

        bv_row = constp.tile([1, D], F32R, tag="bvrow")
        bo_row = constp.tile([1, D], F32R, tag="borow")
        bv_stage = constp.tile([1, D], F32, tag="bstage", bufs=2, name="bv_stage")
        nc.gpsimd.dma_start(bv_stage[:], bv.rearrange("(o d) -> o d", o=1))
        nc.vector.tensor_copy(bv_row[:], bv_stage[:])
        bo_stage = constp.tile([1, D], F32, tag="bstage", bufs=2, name="bo_stage")
        nc.gpsimd.dma_start(bo_stage[:], bo.rearrange("(o d) -> o d", o=1))
        nc.vector.tensor_copy(bo_row[:], bo_stage[:])
        # broadcast bias rows to all partitions via K=1 outer-product matmul
        bv_b = constp.tile([P, D], F32, tag="bvb")
        bo_b = constp.tile([P, D], F32, tag="bob")
        for row, bcast in ((bv_row, bv_b), (bo_row, bo_b)):
            for c0, c1 in ((0, 512), (512, 768)):
                bps = psflow.tile([P, 512], F32, tag="ps", name="bps")
                nc.tensor.matmul(
                    bps[:, : c1 - c0],
                    ones_row[:],
                    row[:, c0:c1],
                    start=True,
                    stop=True,
                )
                nc.vector.tensor_copy(bcast[:, c0:c1], bps[:, : c1 - c0])

        ones_col = constp.tile([P, H], F32, tag="onescol")
        nc.vector.memset(ones_col[:], 1.0)
        ind_stage = constp.tile([H, D], F32, tag="indstage")
        nc.gpsimd.dma_start(ind_stage[:], indsel[:, :])
        ind_t = constp.tile([H, D], F32R, tag="indt")
        nc.vector.tensor_copy(ind_t[:], ind_stage[:])


        # ---- phase A: xT comes pre-transposed from the host ----------------
        xts = [
            xtp.tile([P, S], MMDT, tag=f"xT{mt}", name=f"xT{mt}")
            for mt in range(MT)
        ]
        for mt in range(MT):
            nc.sync.dma_start(
                xts[mt][:, 0:512], xT[mt * P : (mt + 1) * P, 0:512]
            )
            nc.gpsimd.dma_start(
                xts[mt][:, 512:1024], xT[mt * P : (mt + 1) * P, 512:1024]
            )

        # ---- weights (bf16, direct DMA) ------------------------------------
        def load_w(dram, pfx):
            tiles = []
            for mt in range(MT):
                wt = wtsp.tile([P, D], MMDT, tag="w", name=f"{pfx}{mt}")
                nc.gpsimd.dma_start(wt[:], dram[mt * P : (mt + 1) * P, :])
                tiles.append(wt)
            return tiles

        wq_t = load_w(wq, "wq")
        wk_t = load_w(wk, "wk")
        wv_t = load_w(wv, "wv")
        wo_t = load_w(wo, "wo")

        # ---- phase B/C interleaved: per q-half, emit only the projection
        # slices that half needs, then its attention — later projections
        # fill PE gaps while ACT runs exp.
        qts = [qtp.tile([P, S], MMDT, tag=f"qt{i}", name=f"qt{i}") for i in range(MT)]
        kts = [ktp.tile([P, S], MMDT, tag=f"kt{i}", name=f"kt{i}") for i in range(MT)]
        vps = [
            vpp.tile([P, H * 65], MMDT, tag=f"vp{st}", name=f"vp{st}")
            for st in range(NKT)
        ]

        def proj_qk_gen(w_t, b_t, dst, sc, hdb):
            s0 = sc * 512
            ps0 = psflow.tile([P, 512], F32, tag="ps", name="pj0")
            for mt in range(MT):
                nc.tensor.matmul(
                    ps0[:], w_t[mt][:, hdb * P : (hdb + 1) * P],
                    xts[mt][:, s0 : s0 + 512],
                    start=(mt == 0), stop=(mt == MT - 1),
                )
                if mt == 2:
                    yield
            bsl = b_t[:, hdb : hdb + 1]
            bb = bass.AP(bsl.tensor, bsl.offset, [bsl.ap[0], [0, 512]])
            nc.vector.tensor_tensor(
                dst[hdb][:, s0 : s0 + 512], ps0[:], bb, op=add
            )

        def proj_qk_piece(w_t, b_t, dst, sc, hdb):
            for _ in proj_qk_gen(w_t, b_t, dst, sc, hdb):
                pass

        def proj_qk_chunk(w_t, b_t, dst, sc):
            for hdb in range(MT):
                proj_qk_piece(w_t, b_t, dst, sc, hdb)

        def proj_v_gen(st):
            vv = vps[st].rearrange("p (h c) -> p h c", c=65)
            nc.vector.tensor_copy(
                vv[:, :, 64:65],
                ones_col.rearrange("p (h c) -> p h c", c=1),
            )
            ps0 = psflow.tile([P, 512], F32, tag="ps", name="pv0")
            ps1 = psflow.tile([P, 512], F32, tag="ps", name="pv1")
            for mt in range(MT):
                lx = xts[mt][:, st * P : (st + 1) * P]
                nc.tensor.matmul(
                    ps0[:], lx, wv_t[mt][:, 0:512],
                    start=(mt == 0), stop=(mt == MT - 1),
                )
                nc.tensor.matmul(
                    ps1[:, 0:256], lx, wv_t[mt][:, 512:768],
                    start=(mt == 0), stop=(mt == MT - 1),
                )
                if mt in (1, 3):
                    yield
            bsrc = bv_b.rearrange("p (h c) -> p h c", c=DH)
            nc.vector.tensor_tensor(
                vv[:, 0:8, 0:DH],
                ps0.rearrange("p (h c) -> p h c", c=DH),
                bsrc[:, 0:8, :],
                op=add,
            )
            nc.vector.tensor_tensor(
                vv[:, 8:12, 0:DH],
                ps1[:, 0:256].rearrange("p (h c) -> p h c", c=DH),
                bsrc[:, 8:12, :],
                op=add,
            )

        def proj_v(st):
            for _ in proj_v_gen(st):
                pass

        class Feeder:
            """Doles out deferred emission work in ~3-matmul steps so the
            PE stream interleaves finely with attention matmuls."""

            def __init__(self):
                from collections import deque
                self.q = deque()

            def add(self, gen):
                self.q.append(gen)

            def step(self):
                while self.q:
                    try:
                        next(self.q[0])
                        return
                    except StopIteration:
                        self.q.popleft()

            def drain(self):
                while self.q:
                    self.step()

        feeder = Feeder()

        def attn_core(pp, hp):
            q0 = pp * QH
            nkt0 = 4 * pp + 2
            nkt1 = 4 * pp + 4
            pvs = [
                psacc.tile([65, QH], F32, tag="pv", name=f"pv{sub}")
                for sub in range(2)
            ]
            def emit_pv(kt, ests):
                both = kt < nkt0
                c0 = 0 if both else QC
                for sub in range(2):
                    h = 2 * hp + sub
                    nc.tensor.matmul(
                        pvs[sub][:, c0:QH],
                        vps[kt][:, h * 65 : (h + 1) * 65],
                        ests[sub][:, c0:QH],
                        start=(kt == 0),
                        stop=(kt == nkt1 - 1),
                        skip_group_check=True,
                    )

            for kt in range(nkt1):
                both = kt < nkt0
                c0 = 0 if both else QC
                scs = [
                    psflow.tile([P, 512], F32, tag="ps", name=f"sc{sub}")
                    for sub in range(2)
                ]
                # pair sits on disjoint PE row groups -> runs concurrently
                for sub in range(2):
                    r0 = sub * 64
                    nc.tensor.matmul(
                        scs[sub][:, c0:QH],
                        kts[hp][r0 : r0 + 64, kt * P : (kt + 1) * P],
                        qts[hp][r0 : r0 + 64, q0 + c0 : q0 + QH],
                        start=True,
                        stop=True,
                        tile_position=(r0, 0),
                    )
                ests = [expp.tile([P, QH], MMDT, tag="est", name=f"est{sub}")
                        for sub in range(2)]
                for sub in range(2):
                    nc.scalar.activation(
                        ests[sub][:, c0:QH], scs[sub][:, c0:QH], Exp,
                        scale=SCALE,
                    )
                    if kt in (4 * pp, 4 * pp + 1):
                        nc.gpsimd.affine_select(
                            ests[sub][:, 0:QC], ests[sub][:, 0:QC],
                            pattern=[[1, QC]],
                            compare_op=is_ge, fill=0.0,
                            base=(0 if kt == 4 * pp else -P),
                            channel_multiplier=-1,
                        )
                    if kt in (4 * pp + 2, 4 * pp + 3):
                        nc.gpsimd.affine_select(
                            ests[sub][:, QC:QH], ests[sub][:, QC:QH],
                            pattern=[[1, QC]],
                            compare_op=is_ge, fill=0.0,
                            base=(0 if kt == 4 * pp + 2 else -P),
                            channel_multiplier=-1,
                        )
                emit_pv(kt, ests)
            return pvs

        def attn_norm(pp, hp, pvs, wstack):
            # per-pair normalization, deferred one head-pair so the PE
            # stream never waits on the ACT ln/exp chain.
            # 1/r = exp(-ln r): both in the same ACT table set as the
            # softmax Exp (no table reload); reads PSUM rows directly and
            # the Exp write rounds to f32r for the broadcast matmul.
            lnr = smallp.tile([1, 2 * QH], F32, tag="lnr", bufs=3,
                              name=f"lnr{pp}_{hp}")
            for sub in range(2):
                nc.scalar.activation(
                    lnr[:, sub * QH : (sub + 1) * QH], pvs[sub][64:65, :],
                    mybir.ActivationFunctionType.Ln,
                )
            frecr = smallp.tile([1, 2 * QH], F32R, tag="frecr", bufs=3,
                                name=f"frecr{pp}_{hp}")
            nc.scalar.activation(frecr[:], lnr[:], Exp, scale=-1.0)
            rbs = [psflow.tile([P, 512], F32, tag="ps", name=f"rb{sub}")
                   for sub in range(2)]
            for sub in range(2):
                nc.tensor.matmul(
                    rbs[sub][0:64, :], ones_row[:, 0:64],
                    frecr[:, sub * QH : (sub + 1) * QH],
                    start=True, stop=True,
                )
            for sub in range(2):
                r0 = sub * 64
                nc.vector.tensor_copy(
                    wstack[hp][r0 : r0 + 64, :], pvs[sub][0:64, :]
                )
                nc.vector.tensor_tensor(
                    wstack[hp][r0 : r0 + 64, :],
                    wstack[hp][r0 : r0 + 64, :],
                    rbs[sub][0:64, :], op=mult,
                )

        def outproj_gen(pp, wstack, sub):
            q0 = pp * QH
            opsa = psflow.tile([P, 512], F32, tag="ps", name="opa_t")
            opsb = psflow.tile([P, 512], F32, tag="ps", name="opb_t")
            for hdt in range(MT):
                lw = wstack[hdt][:, sub * P : (sub + 1) * P]
                nc.tensor.matmul(
                    opsa[:], lw, wo_t[hdt][:, 0:512],
                    start=(hdt == 0), stop=(hdt == MT - 1),
                )
                nc.tensor.matmul(
                    opsb[:, 0:256], lw, wo_t[hdt][:, 512:768],
                    start=(hdt == 0), stop=(hdt == MT - 1),
                )
                if hdt in (1, 3):
                    yield
            osb = outp.tile([P, D], F32, tag="osb")
            nc.vector.tensor_tensor(
                osb[:, 0:512], opsa[:], bo_b[:, 0:512], op=add
            )
            nc.vector.tensor_tensor(
                osb[:, 512:768], opsb[:, 0:256], bo_b[:, 512:768], op=add
            )
            nc.gpsimd.dma_start(
                y[q0 + sub * P : q0 + (sub + 1) * P, :], osb[:]
            )

        def outproj_sub(pp, wstack, sub):
            for _ in outproj_gen(pp, wstack, sub):
                pass

        # emission order: half-0 projections -> half-0 attention with
        # half-1 projection pieces interleaved between head pairs -> half-1
        # attention with half-0's out-projection interleaved -> tail
        proj_qk_chunk(wq_t, bq_t, qts, 0)
        proj_qk_chunk(wk_t, bk_t, kts, 0)
        for st in range(4):
            proj_v(st)

        wstack0 = [
            wsp.tile([P, QH], MMDT, tag="ws", name=f"ws0_{i}")
            for i in range(MT)
        ]
        wstack1 = [
            wsp.tile([P, QH], MMDT, tag="ws", name=f"ws1_{i}")
            for i in range(MT)
        ]
        prev = None
        for hp in range(MT):
            pvs = attn_core(0, hp)
            if prev is not None:
                attn_norm(0, hp - 1, prev, wstack0)
            prev = pvs
            proj_qk_piece(wq_t, bq_t, qts, 1, hp)
            proj_qk_piece(wk_t, bk_t, kts, 1, hp)
            if hp < 4:
                proj_v(4 + hp)
        attn_norm(0, MT - 1, prev, wstack0)
        prev = None
        for hp in range(MT):
            pvs = attn_core(1, hp)
            if prev is not None:
                attn_norm(1, hp - 1, prev, wstack1)
            prev = pvs
            if hp < 4:
                outproj_sub(0, wstack0, hp)
        attn_norm(1, MT - 1, prev, wstack1)
        for sub in range(4):
            outproj_sub(1, wstack1, sub)
    return nc


_NC_CACHE = None
LAST_EXEC_NS = None


def _indsel() -> np.ndarray:
    """indsel[k, hp*128+m] = 1 where k = 2*hp + (m >= 64): broadcasts each
    head's 1/denominator row via lhsT.T @ recs."""
    ind = np.zeros((H, D), np.float32)
    for hp in range(MT):
        ind[2 * hp, hp * P : hp * P + 64] = 1.0
        ind[2 * hp + 1, hp * P + 64 : (hp + 1) * P] = 1.0
    return ind


def _get_nc():
    global _NC_CACHE
    if _NC_CACHE is None:
        _NC_CACHE = build_nc()
    return _NC_CACHE


def kernel(
    normalized_resid_pre, W_Q, W_K, W_V, W_O, b_Q, b_K, b_V, b_O
) -> np.ndarray:
    global LAST_EXEC_NS
    bf = ml_dtypes.bfloat16
    x = np.asarray(normalized_resid_pre, np.float32)
    xT = np.ascontiguousarray(x.transpose(0, 2, 1)).astype(bf)  # [b, D, S]
    wq = np.asarray(W_Q, np.float32).transpose(1, 0, 2).reshape(D, D).astype(bf)
    wk = np.asarray(W_K, np.float32).transpose(1, 0, 2).reshape(D, D).astype(bf)
    wv = np.asarray(W_V, np.float32).transpose(1, 0, 2).reshape(D, D).astype(bf)
    wo = np.asarray(W_O, np.float32).reshape(D, D).astype(bf)
    bq = np.asarray(b_Q, np.float32).reshape(D).copy()
    bk = np.asarray(b_K, np.float32).reshape(D).copy()
    bv = np.asarray(b_V, np.float32).reshape(D).copy()
    bo = np.asarray(b_O, np.float32).reshape(D).copy()
    ind = _indsel()

    nc = _get_nc()
    in_maps = [
        {
            "xT": xT[i],
            "wq": wq, "wk": wk, "wv": wv, "wo": wo,
            "bq": bq, "bk": bk, "bv": bv, "bo": bo, "indsel": ind,
        }
        for i in range(N_CORES)
    ]
    trace = os.environ.get("KERNEL_TRACE", "0") == "1"
    res = run_bass_kernel_spmd(
        nc, in_maps, list(range(N_CORES)), trace=trace
    )
    LAST_EXEC_NS = res.exec_time_ns
    out = np.stack(
        [res.results[i]["y"].astype(np.float32) for i in range(N_CORES)], axis=0
    )
    return out
